# revision 14
# baseline (speedup 1.0000x reference)
"""TRN2 Bass kernel for nn_ClassNetPP (retrieval_knn).

Pipeline per image (channel-major activations [C, N] on-chip, N = H*W = 1024):
  adapter 1x1 conv + BN + ReLU -> ResidualContextBlock (1x1 reduce, 3x3 conv,
  3x3 dilated conv, 1x1 project + residual) -> final 1x1 conv + bias ->
  cosine sims vs 1280 prototypes -> max over K -> act maps + logits,
  plus L2-normalized embeddings.

Sharding: data-parallel over batch B=16 across 8 NeuronCores (2 images/core).
All matmuls run as fp32r (full PE rate at moving-dim >= 256, ~1e-4 rel err).
BN is folded into conv weights + per-channel bias on the host. The host also
pre-transposes inputs/weights so the device only does dense matmul-shaped work.
"""
import numpy as np

import concourse.bacc as bacc
import concourse.bass as bass
import concourse.mybir as mybir
import concourse.tile as tile
from concourse.bass_utils import run_bass_kernel_spmd
from concourse.masks import make_identity

F32 = mybir.dt.float32
F32R = mybir.dt.float32r

EPS = 1e-5
B, N, DIN, D, HID, C, K = 16, 1024, 768, 512, 128, 20, 64
CK = C * K  # 1280
NCORES = 8
BPC = B // NCORES  # images per core
KT0 = DIN // 128   # 6  k-tiles for adapter
MT = D // 128      # 4  m-tiles for D=512


def _r(ap):
    return ap.bitcast(F32R)


def _f(ap):
    return ap.bitcast(F32)


def _emit_image(nc, tc, pools, w, b_idx, dram, xt_sb=None):
    """Emit one image's pipeline. w: dict of SBUF weight APs; dram: dict of DRAM APs."""
    sb_big, sb_small, sb_scr, sb_rows, xt_pool, psA, ps_q, ps_b = pools

    # ---- load transposed input [768, 1024] as 6 k-tiles (image 0's is preloaded)
    if xt_sb is None:
        xt_sb = xt_pool.tile([128, KT0, N], F32R, name="xt_sb")
        nc.sync.dma_start(out=xt_sb, in_=dram["xt"][b_idx].rearrange("(kt p) n -> p kt n", p=128))

    # ---- L0: y0 = relu(w0t.T @ xt + t0)   [512, 1024]
    y0 = sb_big.tile([128, MT, N], F32R, name="y0")
    for mt in range(MT):
        for ch in range(2):
            ps = psA.tile([128, 512], F32, name="ps", tag="psA")
            for k in range(KT0):
                nc.tensor.matmul(
                    ps, w["w0t"][:, k, mt * 128:(mt + 1) * 128],
                    xt_sb[:, k, ch * 512:(ch + 1) * 512],
                    start=(k == 0), stop=(k == KT0 - 1),
                )
            nc.scalar.activation(
                out=y0[:, mt, ch * 512:(ch + 1) * 512], in_=ps,
                func=mybir.ActivationFunctionType.Relu,
                bias=w["t0"][:, mt:mt + 1], scale=1.0,
            )

    # ---- Lr: o = relu(wrt.T @ y0 + tr) -> zero-padded [128, 36, 36]
    o_pad = sb_small.tile([128, 36, 36], F32R, name="o_pad")
    # zero the padding border via DMA from a zeros constant (DMA may write f32r)
    zb = dram["zeros_d"]
    nc.sync.dma_start(out=o_pad[:, 0:2, :], in_=zb[0:72].partition_broadcast(128))
    nc.sync.dma_start(out=o_pad[:, 34:36, :], in_=zb[0:72].partition_broadcast(128))
    nc.sync.dma_start(out=o_pad[:, 2:34, 0:2], in_=zb[0:64].partition_broadcast(128))
    nc.sync.dma_start(out=o_pad[:, 2:34, 34:36], in_=zb[0:64].partition_broadcast(128))
    for ch in range(2):
        ps = psA.tile([128, 512], F32, name="ps", tag="psA")
        for k in range(MT):
            nc.tensor.matmul(
                ps, w["wrt"][:, k, :], y0[:, k, ch * 512:(ch + 1) * 512],
                start=(k == 0), stop=(k == MT - 1),
            )
        # interior rows [2+16*ch : 2+16*(ch+1)], cols [2:34]
        nc.scalar.activation(
            out=o_pad[:, 2 + 16 * ch:2 + 16 * (ch + 1), 2:34], in_=ps,
            func=mybir.ActivationFunctionType.Relu,
            bias=w["tr"], scale=1.0,
        )

    # ---- 3x3 convs: l (dil=1, base offset 1) and g (dil=2, base offset 0)
    conv_outs = {}
    for cname, wname, bname, base, dil in (
        ("l_t", "wlt", "tl", 1, 1),
        ("g_t", "wgt", "tg", 0, 2),
    ):
        out_t = sb_small.tile([128, N], F32R, name=cname)
        for hc in range(2):  # 16 output rows per chunk
            ps = psA.tile([128, 512], F32, name="ps", tag="psA")
            for tap in range(9):
                dh, dw = tap // 3, tap % 3
                oh = base + dh * dil + hc * 16
                ow = base + dw * dil
                nc.tensor.matmul(
                    ps, w[wname][:, tap, :],
                    o_pad[:, oh:oh + 16, ow:ow + 32],
                    start=(tap == 0), stop=(tap == 8),
                )
            nc.scalar.activation(
                out=out_t[:, hc * 512:(hc + 1) * 512], in_=ps,
                func=mybir.ActivationFunctionType.Relu,
                bias=w[bname], scale=1.0,
            )
        conv_outs[cname] = out_t
    l_t, g_t = conv_outs["l_t"], conv_outs["g_t"]

    # ---- Lp + residual: x1 = relu(wpt.T @ [l;g] + tp + y0)
    x1 = sb_big.tile([128, MT, N], F32R, name="x1")
    for mt in range(MT):
        for ch in range(2):
            ps = psA.tile([128, 512], F32, name="ps", tag="psA")
            nc.tensor.matmul(ps, w["wpt"][:, 0, mt * 128:(mt + 1) * 128],
                             l_t[:, ch * 512:(ch + 1) * 512], start=True, stop=False)
            nc.tensor.matmul(ps, w["wpt"][:, 1, mt * 128:(mt + 1) * 128],
                             g_t[:, ch * 512:(ch + 1) * 512], start=False, stop=True)
            sl = x1[:, mt, ch * 512:(ch + 1) * 512]
            nc.vector.scalar_tensor_tensor(
                out=sl, in0=ps, scalar=w["tp"][:, mt:mt + 1],
                in1=_f(y0[:, mt, ch * 512:(ch + 1) * 512]),
                op0=mybir.AluOpType.add, op1=mybir.AluOpType.add,
            )
            nc.scalar.activation(out=sl, in_=_f(sl), func=mybir.ActivationFunctionType.Relu)

    # ---- Lf: xf = wft.T @ x1 + bf, with the ssq row-matmuls interleaved so
    # the PE never waits on the DVE square pass (pq accumulation groups span
    # the mt loop; other-bank matmuls interleave, which the HW allows).
    xf = sb_big.tile([128, MT, N], F32R, name="xf")
    sq = sb_scr.tile([128, MT, N], F32R, name="sq", tag="scr")
    pq0 = ps_q.tile([2, 512], F32, name="pq0", tag="pq")
    pq1 = ps_q.tile([2, 512], F32, name="pq1", tag="pq")
    pqs = (pq0, pq1)
    for mt in range(MT):
        for ch in range(2):
            ps = psA.tile([128, 512], F32, name="ps", tag="psA")
            for k in range(MT):
                nc.tensor.matmul(
                    ps, w["wft"][:, k, mt * 128:(mt + 1) * 128],
                    x1[:, k, ch * 512:(ch + 1) * 512],
                    start=(k == 0), stop=(k == MT - 1),
                )
            nc.scalar.activation(
                out=xf[:, mt, ch * 512:(ch + 1) * 512], in_=ps,
                func=mybir.ActivationFunctionType.Identity,
                bias=w["bf"][:, mt:mt + 1], scale=1.0,
            )
        nc.vector.tensor_mul(out=sq[:, mt, :], in0=_f(xf[:, mt, :]), in1=_f(xf[:, mt, :]))
        for ch in range(2):
            nc.tensor.matmul(
                pqs[ch], w["ones_r"], sq[:, mt, ch * 512:(ch + 1) * 512],
                start=(mt == 0), stop=(mt == MT - 1), skip_group_check=True,
            )

    # ---- rnorm_row = 1/max(sqrt(ssq), 1e-12) as [1, 1024] at partition 0
    srt = sb_rows.tile([1, N], F32, name="srt")
    for ch in range(2):
        nc.scalar.activation(out=srt[0:1, ch * 512:(ch + 1) * 512], in_=pqs[ch][0:1, :],
                             func=mybir.ActivationFunctionType.Sqrt)
    nc.vector.tensor_scalar_max(out=srt, in0=srt, scalar1=1e-12)
    nc.vector.reciprocal(out=srt, in_=srt)
    rnr = sb_rows.tile([1, N], F32R, name="rnr")
    nc.vector.tensor_copy(out=rnr, in_=srt)

    # ---- sims loop with interleaved broadcast / act-map transposes / x_norm
    # writeback, so every PE instruction's inputs are long since ready.
    a_nm = sb_small.tile([128, 8, C], F32, name="a_nm")
    amap_sb = sb_small.tile([C, N], F32, name="amap_sb")
    bcsb = sb_rows.tile([128, N], F32, name="bcsb")
    xo = sb_scr.tile([128, MT, N], F32, name="xo", tag="scr")
    chunks = [(0, 512, 8), (512, 512, 8), (1024, 256, 4)]  # (ck0, width, nclasses)

    def sims_nt(nt):
        c0 = 0
        for ck0, width, ncl in chunks:
            ps = psA.tile([128, 512], F32, name="ps", tag="psA")
            for k in range(MT):
                nc.tensor.matmul(
                    ps[:, :width], xf[:, k, nt * 128:(nt + 1) * 128],
                    w["pt"][:, k, ck0:ck0 + width],
                    start=(k == 0), stop=(k == MT - 1),
                )
            nc.vector.reduce_max(
                out=a_nm[:, nt, c0:c0 + ncl],
                in_=ps[:, :width].rearrange("p (c k) -> p c k", k=K),
                axis=mybir.AxisListType.X,
            )
            c0 += ncl

    def transpose_nt(nt):
        pst = psA.tile([128, 512], F32, name="ps", tag="psA")
        nc.tensor.transpose(pst[:C, :128], a_nm[:, nt, :], w["ident"])
        nc.scalar.copy(out=amap_sb[:, nt * 128:(nt + 1) * 128], in_=pst[:C, :128])

    sims_nt(0)
    # broadcast rnorm across partitions (rnorm chain finished under sims 0)
    for ch in range(2):
        bc = ps_b.tile([128, 512], F32, name="bc", tag="bc")
        nc.tensor.matmul(bc, w["ones_row"], rnr[0:1, ch * 512:(ch + 1) * 512],
                         start=True, stop=True)
        nc.scalar.copy(out=bcsb[:, ch * 512:(ch + 1) * 512], in_=bc)
    for nt in range(1, 8):
        sims_nt(nt)
        transpose_nt(nt - 1)
        if nt % 2 == 1:
            mt = (nt - 1) // 2
            nc.gpsimd.tensor_mul(out=xo[:, mt, :], in0=_f(xf[:, mt, :]), in1=bcsb)
            nc.sync.dma_start(out=dram["xout"][b_idx, mt * 128:(mt + 1) * 128, :],
                              in_=xo[:, mt, :])
    transpose_nt(7)

    # ---- act-map scale by rnorm rows, logits, store
    for ch in range(2):
        nc.vector.tensor_mul(out=amap_sb[:, ch * 512:(ch + 1) * 512],
                             in0=amap_sb[:, ch * 512:(ch + 1) * 512],
                             in1=bcsb[0:C, ch * 512:(ch + 1) * 512])
    nc.sync.dma_start(out=dram["amap"][b_idx], in_=amap_sb)
    lg = sb_small.tile([C, 1], F32, name="lg")
    nc.vector.reduce_max(out=lg, in_=amap_sb, axis=mybir.AxisListType.X)
    nc.sync.dma_start(out=dram["logit"][b_idx], in_=lg)


def _build_program():
    nc = bacc.Bacc("TRN2", target_bir_lowering=False, debug=False, num_devices=NCORES)

    dram = {
        "xt": nc.dram_tensor("xt", [BPC, DIN, N], F32R, kind="ExternalInput").ap(),
        "w0t_d": nc.dram_tensor("w0t", [DIN, D], F32R, kind="ExternalInput").ap(),
        "wrt_d": nc.dram_tensor("wrt", [D, HID], F32R, kind="ExternalInput").ap(),
        "wlt_d": nc.dram_tensor("wlt", [9, HID, HID], F32R, kind="ExternalInput").ap(),
        "wgt_d": nc.dram_tensor("wgt", [9, HID, HID], F32R, kind="ExternalInput").ap(),
        "wpt_d": nc.dram_tensor("wpt", [2 * HID, D], F32R, kind="ExternalInput").ap(),
        "wft_d": nc.dram_tensor("wft", [D, D], F32R, kind="ExternalInput").ap(),
        "pt_d": nc.dram_tensor("pt", [D, CK], F32R, kind="ExternalInput").ap(),
        "t0_d": nc.dram_tensor("t0", [D], F32, kind="ExternalInput").ap(),
        "tr_d": nc.dram_tensor("tr", [HID], F32, kind="ExternalInput").ap(),
        "tl_d": nc.dram_tensor("tl", [HID], F32, kind="ExternalInput").ap(),
        "tg_d": nc.dram_tensor("tg", [HID], F32, kind="ExternalInput").ap(),
        "tp_d": nc.dram_tensor("tp", [D], F32, kind="ExternalInput").ap(),
        "bf_d": nc.dram_tensor("bf", [D], F32, kind="ExternalInput").ap(),
        "ones_d": nc.dram_tensor("ones", [256], F32R, kind="ExternalInput").ap(),
        "zeros_d": nc.dram_tensor("zeros", [72], F32R, kind="ExternalInput").ap(),
        "xout": nc.dram_tensor("xout", [BPC, D, N], F32, kind="ExternalOutput").ap(),
        "amap": nc.dram_tensor("amap", [BPC, C, N], F32, kind="ExternalOutput").ap(),
        "logit": nc.dram_tensor("logit", [BPC, C], F32, kind="ExternalOutput").ap(),
    }

    with tile.TileContext(nc) as tc:
        with (
            tc.tile_pool(name="wpool", bufs=1) as wpool,
            tc.tile_pool(name="xt_pool", bufs=1) as xt_pool,
            tc.tile_pool(name="sb_big", bufs=1) as sb_big,
            tc.tile_pool(name="sb_small", bufs=2) as sb_small,
            tc.tile_pool(name="sb_scr", bufs=1) as sb_scr,
            tc.tile_pool(name="sb_rows", bufs=1) as sb_rows,
            tc.tile_pool(name="psA", bufs=5, space="PSUM") as psA,
            tc.tile_pool(name="ps_q", bufs=2, space="PSUM") as ps_q,
            tc.tile_pool(name="ps_b", bufs=1, space="PSUM") as ps_b,
        ):
            w = {}
            # critical-path loads first: adapter weight + bias + image-0 input
            w["w0t"] = wpool.tile([128, KT0, D], F32R, name="w0t_sb")
            nc.sync.dma_start(out=w["w0t"], in_=dram["w0t_d"].rearrange("(kt p) m -> p kt m", p=128))
            w["t0"] = wpool.tile([128, MT], F32, name="t0_sb")
            nc.sync.dma_start(out=w["t0"], in_=dram["t0_d"].rearrange("(mt p) -> p mt", p=128))
            xt0 = xt_pool.tile([128, KT0, N], F32R, name="xt_sb")
            nc.sync.dma_start(out=xt0, in_=dram["xt"][0].rearrange("(kt p) n -> p kt n", p=128))

            # remaining weights load under image-0's adapter compute
            w["wrt"] = wpool.tile([128, MT, HID], F32R, name="wrt_sb")
            nc.sync.dma_start(out=w["wrt"], in_=dram["wrt_d"].rearrange("(kt p) m -> p kt m", p=128))
            w["wlt"] = wpool.tile([128, 9, HID], F32R, name="wlt_sb")
            nc.sync.dma_start(out=w["wlt"], in_=dram["wlt_d"].rearrange("t p m -> p t m"))
            w["wgt"] = wpool.tile([128, 9, HID], F32R, name="wgt_sb")
            nc.sync.dma_start(out=w["wgt"], in_=dram["wgt_d"].rearrange("t p m -> p t m"))
            w["wpt"] = wpool.tile([128, 2, D], F32R, name="wpt_sb")
            nc.sync.dma_start(out=w["wpt"], in_=dram["wpt_d"].rearrange("(kt p) m -> p kt m", p=128))
            w["wft"] = wpool.tile([128, MT, D], F32R, name="wft_sb")
            nc.sync.dma_start(out=w["wft"], in_=dram["wft_d"].rearrange("(kt p) m -> p kt m", p=128))
            w["pt"] = wpool.tile([128, MT, CK], F32R, name="pt_sb")
            nc.sync.dma_start(out=w["pt"], in_=dram["pt_d"].rearrange("(kt p) m -> p kt m", p=128))
            for nm, srcn, width in (("tp", "tp_d", MT), ("bf", "bf_d", MT),
                                    ("tr", "tr_d", 1), ("tl", "tl_d", 1), ("tg", "tg_d", 1)):
                w[nm] = wpool.tile([128, width], F32, name=f"{nm}_sb")
                nc.sync.dma_start(out=w[nm], in_=dram[srcn].rearrange("(mt p) -> p mt", p=128))
            w["ident"] = wpool.tile([128, 128], F32, name="ident")
            make_identity(nc, w["ident"])
            w["ones_r"] = wpool.tile([128, 2], F32R, name="ones_r")
            nc.sync.dma_start(out=w["ones_r"], in_=dram["ones_d"].rearrange("(p two) -> p two", two=2))
            w["ones_row"] = wpool.tile([1, 128], F32R, name="ones_row")
            nc.sync.dma_start(out=w["ones_row"], in_=dram["ones_d"][0:128].rearrange("(one p) -> one p", one=1))

            pools = (sb_big, sb_small, sb_scr, sb_rows, xt_pool, psA, ps_q, ps_b)
            _emit_image(nc, tc, pools, w, 0, dram, xt_sb=xt0)
            for b_idx in range(1, BPC):
                _emit_image(nc, tc, pools, w, b_idx, dram)

    nc.compile()
    return nc


_PROG = None


def _get_program():
    global _PROG
    if _PROG is None:
        _PROG = _build_program()
    return _PROG


def _fold(wc, g, b, m, v):
    s = g / np.sqrt(v + EPS)
    return wc * s[:, None, None, None], (b - m * s).astype(np.float32)


def kernel(patch_feats, protos, w0, g0, b0, m0, v0, wr, gr, br, mr, vr,
           wl, gl, bl, ml, vl, wg, gg, bg, mg, vg, wp, gp, bp, mp, vp,
           wf, bf, logit_scale):
    patch_feats = np.asarray(patch_feats, np.float32)
    f32 = lambda x: np.asarray(x, np.float32)

    w0f, t0 = _fold(f32(w0), f32(g0), f32(b0), f32(m0), f32(v0))
    wrf, tr = _fold(f32(wr), f32(gr), f32(br), f32(mr), f32(vr))
    wlf, tl = _fold(f32(wl), f32(gl), f32(bl), f32(ml), f32(vl))
    wgf, tg = _fold(f32(wg), f32(gg), f32(bg), f32(mg), f32(vg))
    wpf, tp = _fold(f32(wp), f32(gp), f32(bp), f32(mp), f32(vp))

    weights = {
        "w0t": np.ascontiguousarray(w0f[:, :, 0, 0].T),
        "wrt": np.ascontiguousarray(wrf[:, :, 0, 0].T),
        "wlt": np.ascontiguousarray(wlf.transpose(2, 3, 1, 0).reshape(9, HID, HID)),
        "wgt": np.ascontiguousarray(wgf.transpose(2, 3, 1, 0).reshape(9, HID, HID)),
        "wpt": np.ascontiguousarray(wpf[:, :, 0, 0].T),
        "wft": np.ascontiguousarray(f32(wf)[:, :, 0, 0].T),
        "pt": np.ascontiguousarray(f32(protos).reshape(CK, D).T),
        "t0": t0, "tr": tr, "tl": tl, "tg": tg, "tp": tp,
        "bf": f32(bf),
        "ones": np.ones(256, np.float32),
        "zeros": np.zeros(72, np.float32),
    }
    xt_all = np.ascontiguousarray(patch_feats.transpose(0, 2, 1))  # [B, 768, 1024]

    nc = _get_program()
    in_maps = []
    for c in range(NCORES):
        im = {"xt": xt_all[c * BPC:(c + 1) * BPC]}
        im.update(weights)
        in_maps.append(im)
    res = run_bass_kernel_spmd(nc, in_maps, list(range(NCORES)))

    logits = np.concatenate([res.results[c]["logit"] for c in range(NCORES)], axis=0)
    amap = np.concatenate([res.results[c]["amap"] for c in range(NCORES)], axis=0)
    x_cm = np.concatenate([res.results[c]["xout"] for c in range(NCORES)], axis=0)

    logits = (logits * float(np.asarray(logit_scale))).astype(np.float32)
    act_maps = amap.reshape(B, C, 32, 32).astype(np.float32)
    x = np.ascontiguousarray(x_cm.transpose(0, 2, 1)).astype(np.float32)
    return logits, act_maps, x


# revision 24
# speedup vs baseline: 1.1919x; 1.1919x over previous
"""TRN2 Bass kernel for nn_ClassNetPP (retrieval_knn).

Pipeline per image (channel-major activations [C, N] on-chip, N = H*W = 1024):
  adapter 1x1 conv + BN + ReLU -> ResidualContextBlock (1x1 reduce, 3x3 conv,
  3x3 dilated conv, 1x1 project + residual) -> final 1x1 conv + bias ->
  cosine sims vs 1280 prototypes -> max over K -> act maps + logits,
  plus L2-normalized embeddings.

Sharding: data-parallel over batch B=16 across 8 NeuronCores (2 images/core).
All matmuls run as fp32r (full PE rate at moving-dim >= 256, ~1e-4 rel err).
BN is folded into conv weights + per-channel bias on the host. The host also
pre-transposes inputs/weights so the device only does dense matmul-shaped work.
"""
import numpy as np

import concourse.bacc as bacc
import concourse.bass as bass
import concourse.mybir as mybir
import concourse.tile as tile
from concourse.bass_utils import run_bass_kernel_spmd
from concourse.masks import make_identity

F32 = mybir.dt.float32
F32R = mybir.dt.float32r

EPS = 1e-5
B, N, DIN, D, HID, C, K = 16, 1024, 768, 512, 128, 20, 64
CK = C * K  # 1280
NCORES = 8
BPC = B // NCORES  # images per core
KT0 = DIN // 128   # 6  k-tiles for adapter
MT = D // 128      # 4  m-tiles for D=512


def _r(ap):
    return ap.bitcast(F32R)


def _f(ap):
    return ap.bitcast(F32)


def _load_xt(nc, dram, b_idx, xt_sb):
    for ch in range(2):
        for k in range(KT0):
            nc.sync.dma_start(
                out=xt_sb[:, k, ch * 512:(ch + 1) * 512],
                in_=dram["xt"][b_idx, k * 128:(k + 1) * 128, ch * 512:(ch + 1) * 512],
            )


def _emit_image(nc, tc, pools, w, b_idx, dram, xt_sb=None):
    """Emit one image's pipeline. w: dict of SBUF weight APs; dram: dict of DRAM APs."""
    sb_big, sb_small, sb_scr, sb_rows, xt_pool, psA, ps_q, ps_b = pools

    # ---- load transposed input [768, 1024] as 6 k-tiles (image 0's is preloaded)
    if xt_sb is None:
        xt_sb = xt_pool.tile([128, KT0, N], F32R, name="xt_sb")
        _load_xt(nc, dram, b_idx, xt_sb)

    # ---- L0: y0 = relu(w0t.T @ xt + t0)   [512, 1024]
    y0 = sb_big.tile([128, MT, N], F32R, name="y0")
    for ch in range(2):
        # k-outer over 4 live accumulators: the PE starts on k-tile 0 as soon
        # as its DMA lands instead of waiting for the whole input
        pss = [psA.tile([128, 512], F32, name="ps", tag="psA") for _ in range(MT)]
        for k in range(KT0):
            for mt in range(MT):
                nc.tensor.matmul(
                    pss[mt], w["w0t"][:, k, mt * 128:(mt + 1) * 128],
                    xt_sb[:, k, ch * 512:(ch + 1) * 512],
                    start=(k == 0), stop=(k == KT0 - 1), skip_group_check=True,
                )
        for mt in range(MT):
            nc.scalar.activation(
                out=y0[:, mt, ch * 512:(ch + 1) * 512], in_=pss[mt],
                func=mybir.ActivationFunctionType.Relu,
                bias=w["t0"][:, mt:mt + 1], scale=1.0,
            )

    # ---- Lr: o = relu(wrt.T @ y0 + tr) -> zero-padded [128, 36, 36]
    o_pad = sb_small.tile([128, 36, 36], F32R, name="o_pad")
    # zero the padding border via DMA from a zeros constant (DMA may write f32r)
    zb = dram["zeros_d"]
    nc.gpsimd.dma_start(out=o_pad[:, 0:2, :], in_=zb[0:72].partition_broadcast(128))
    nc.gpsimd.dma_start(out=o_pad[:, 34:36, :], in_=zb[0:72].partition_broadcast(128))
    nc.gpsimd.dma_start(out=o_pad[:, 2:34, 0:2], in_=zb[0:64].partition_broadcast(128))
    nc.gpsimd.dma_start(out=o_pad[:, 2:34, 34:36], in_=zb[0:64].partition_broadcast(128))
    for ch in range(2):
        ps = psA.tile([128, 512], F32, name="ps", tag="psA")
        for k in range(MT):
            nc.tensor.matmul(
                ps, w["wrt"][:, k, :], y0[:, k, ch * 512:(ch + 1) * 512],
                start=(k == 0), stop=(k == MT - 1),
            )
        # interior rows [2+16*ch : 2+16*(ch+1)], cols [2:34]
        nc.scalar.activation(
            out=o_pad[:, 2 + 16 * ch:2 + 16 * (ch + 1), 2:34], in_=ps,
            func=mybir.ActivationFunctionType.Relu,
            bias=w["tr"], scale=1.0,
        )

    # ---- 3x3 convs: l (dil=1, base offset 1) and g (dil=2, base offset 0)
    conv_outs = {}
    for cname, wname, bname, base, dil in (
        ("l_t", "wlt", "tl", 1, 1),
        ("g_t", "wgt", "tg", 0, 2),
    ):
        out_t = sb_small.tile([128, N], F32R, name=cname)
        for hc in range(2):  # 16 output rows per chunk
            ps = psA.tile([128, 512], F32, name="ps", tag="psA")
            for tap in range(9):
                dh, dw = tap // 3, tap % 3
                oh = base + dh * dil + hc * 16
                ow = base + dw * dil
                nc.tensor.matmul(
                    ps, w[wname][:, tap, :],
                    o_pad[:, oh:oh + 16, ow:ow + 32],
                    start=(tap == 0), stop=(tap == 8),
                )
            nc.scalar.activation(
                out=out_t[:, hc * 512:(hc + 1) * 512], in_=ps,
                func=mybir.ActivationFunctionType.Relu,
                bias=w[bname], scale=1.0,
            )
        conv_outs[cname] = out_t
    l_t, g_t = conv_outs["l_t"], conv_outs["g_t"]

    # ---- Lp + residual: x1 = relu(wpt.T @ [l;g] + tp + y0)
    x1 = sb_big.tile([128, MT, N], F32R, name="x1")
    for mt in range(MT):
        for ch in range(2):
            ps = psA.tile([128, 512], F32, name="ps", tag="psA")
            nc.tensor.matmul(ps, w["wpt"][:, 0, mt * 128:(mt + 1) * 128],
                             l_t[:, ch * 512:(ch + 1) * 512], start=True, stop=False)
            nc.tensor.matmul(ps, w["wpt"][:, 1, mt * 128:(mt + 1) * 128],
                             g_t[:, ch * 512:(ch + 1) * 512], start=False, stop=True)
            sl = x1[:, mt, ch * 512:(ch + 1) * 512]
            nc.vector.scalar_tensor_tensor(
                out=sl, in0=ps, scalar=w["tp"][:, mt:mt + 1],
                in1=_f(y0[:, mt, ch * 512:(ch + 1) * 512]),
                op0=mybir.AluOpType.add, op1=mybir.AluOpType.add,
            )
            nc.scalar.activation(out=sl, in_=_f(sl), func=mybir.ActivationFunctionType.Relu)

    # ---- Lf: xf = wft.T @ x1 + bf, with the ssq row-matmuls interleaved so
    # the PE never waits on the DVE square pass (pq accumulation groups span
    # the mt loop; other-bank matmuls interleave, which the HW allows).
    xf = sb_big.tile([128, MT, N], F32R, name="xf")
    sq = sb_scr.tile([128, MT, N], F32R, name="sq", tag="scr")
    pq0 = ps_q.tile([2, 512], F32, name="pq0", tag="pq")
    pq1 = ps_q.tile([2, 512], F32, name="pq1", tag="pq")
    pqs = (pq0, pq1)
    for mt in range(MT):
        for ch in range(2):
            ps = psA.tile([128, 512], F32, name="ps", tag="psA")
            for k in range(MT):
                nc.tensor.matmul(
                    ps, w["wft"][:, k, mt * 128:(mt + 1) * 128],
                    x1[:, k, ch * 512:(ch + 1) * 512],
                    start=(k == 0), stop=(k == MT - 1),
                )
            nc.scalar.activation(
                out=xf[:, mt, ch * 512:(ch + 1) * 512], in_=ps,
                func=mybir.ActivationFunctionType.Identity,
                bias=w["bf"][:, mt:mt + 1], scale=1.0,
            )
        nc.vector.tensor_mul(out=sq[:, mt, :], in0=_f(xf[:, mt, :]), in1=_f(xf[:, mt, :]))
        for ch in range(2):
            nc.tensor.matmul(
                pqs[ch], w["ones_r"], sq[:, mt, ch * 512:(ch + 1) * 512],
                start=(mt == 0), stop=(mt == MT - 1), skip_group_check=True,
            )

    # ---- rnorm_row = 1/max(sqrt(ssq), 1e-12) as [1, 1024] at partition 0
    srt = sb_rows.tile([1, N], F32, name="srt")
    for ch in range(2):
        nc.scalar.activation(out=srt[0:1, ch * 512:(ch + 1) * 512], in_=pqs[ch][0:1, :],
                             func=mybir.ActivationFunctionType.Sqrt)
    nc.vector.tensor_scalar_max(out=srt, in0=srt, scalar1=1e-12)
    nc.vector.reciprocal(out=srt, in_=srt)
    rnr = sb_rows.tile([1, N], F32R, name="rnr")
    nc.vector.tensor_copy(out=rnr, in_=srt)

    # ---- sims loop with interleaved broadcast / act-map transposes / x_norm
    # writeback, so every PE instruction's inputs are long since ready.
    a_nm = sb_small.tile([128, 8, C], F32, name="a_nm")
    amap_sb = sb_small.tile([C, N], F32, name="amap_sb")
    bcsb = sb_rows.tile([128, N], F32, name="bcsb")
    xo = sb_scr.tile([128, MT, N], F32, name="xo", tag="scr")
    chunks = [(0, 512, 8), (512, 512, 8), (1024, 256, 4)]  # (ck0, width, nclasses)

    def sims_nt(nt):
        c0 = 0
        for ck0, width, ncl in chunks:
            ps = psA.tile([128, 512], F32, name="ps", tag="psA")
            for k in range(MT):
                nc.tensor.matmul(
                    ps[:, :width], xf[:, k, nt * 128:(nt + 1) * 128],
                    w["pt"][:, k, ck0:ck0 + width],
                    start=(k == 0), stop=(k == MT - 1),
                )
            nc.vector.reduce_max(
                out=a_nm[:, nt, c0:c0 + ncl],
                in_=ps[:, :width].rearrange("p (c k) -> p c k", k=K),
                axis=mybir.AxisListType.X,
            )
            c0 += ncl

    def transpose_nt(nt):
        pst = psA.tile([128, 512], F32, name="ps", tag="psA")
        nc.tensor.transpose(pst[:C, :128], a_nm[:, nt, :], w["ident"])
        nc.scalar.copy(out=amap_sb[:, nt * 128:(nt + 1) * 128], in_=pst[:C, :128])

    sims_nt(0)
    # broadcast rnorm across partitions (rnorm chain finished under sims 0)
    for ch in range(2):
        bc = ps_b.tile([128, 512], F32, name="bc", tag="bc")
        nc.tensor.matmul(bc, w["ones_row"], rnr[0:1, ch * 512:(ch + 1) * 512],
                         start=True, stop=True)
        nc.scalar.copy(out=bcsb[:, ch * 512:(ch + 1) * 512], in_=bc)
    for nt in range(1, 8):
        sims_nt(nt)
        if 1 <= nt <= 4:
            mt = nt - 1
            nc.gpsimd.tensor_mul(out=xo[:, mt, :], in0=_f(xf[:, mt, :]), in1=bcsb)
            nc.sync.dma_start(out=dram["xout"][b_idx, mt * 128:(mt + 1) * 128, :],
                              in_=xo[:, mt, :])

    # raw act-map values (n-major) + rnorm row; the host applies the scale,
    # transposes to [C, H, W], and takes the max over n for logits
    nc.sync.dma_start(out=dram["araw"][b_idx], in_=a_nm)
    nc.sync.dma_start(out=dram["rnrow"][b_idx], in_=srt)


def _build_program():
    nc = bacc.Bacc("TRN2", target_bir_lowering=False, debug=False, num_devices=NCORES)

    dram = {
        "xt": nc.dram_tensor("xt", [BPC, DIN, N], F32R, kind="ExternalInput").ap(),
        "w0t_d": nc.dram_tensor("w0t", [DIN, D], F32R, kind="ExternalInput").ap(),
        "wrt_d": nc.dram_tensor("wrt", [D, HID], F32R, kind="ExternalInput").ap(),
        "wlt_d": nc.dram_tensor("wlt", [9, HID, HID], F32R, kind="ExternalInput").ap(),
        "wgt_d": nc.dram_tensor("wgt", [9, HID, HID], F32R, kind="ExternalInput").ap(),
        "wpt_d": nc.dram_tensor("wpt", [2 * HID, D], F32R, kind="ExternalInput").ap(),
        "wft_d": nc.dram_tensor("wft", [D, D], F32R, kind="ExternalInput").ap(),
        "pt_d": nc.dram_tensor("pt", [D, CK], F32R, kind="ExternalInput").ap(),
        "t0_d": nc.dram_tensor("t0", [D], F32, kind="ExternalInput").ap(),
        "tr_d": nc.dram_tensor("tr", [HID], F32, kind="ExternalInput").ap(),
        "tl_d": nc.dram_tensor("tl", [HID], F32, kind="ExternalInput").ap(),
        "tg_d": nc.dram_tensor("tg", [HID], F32, kind="ExternalInput").ap(),
        "tp_d": nc.dram_tensor("tp", [D], F32, kind="ExternalInput").ap(),
        "bf_d": nc.dram_tensor("bf", [D], F32, kind="ExternalInput").ap(),
        "ones_d": nc.dram_tensor("ones", [256], F32R, kind="ExternalInput").ap(),
        "zeros_d": nc.dram_tensor("zeros", [72], F32R, kind="ExternalInput").ap(),
        "xout": nc.dram_tensor("xout", [BPC, D, N], F32, kind="ExternalOutput").ap(),
        "araw": nc.dram_tensor("araw", [BPC, 128, 8, C], F32, kind="ExternalOutput").ap(),
        "rnrow": nc.dram_tensor("rnrow", [BPC, 1, N], F32, kind="ExternalOutput").ap(),
    }

    with tile.TileContext(nc) as tc:
        with (
            tc.tile_pool(name="wpool", bufs=1) as wpool,
            tc.tile_pool(name="xt_pool", bufs=1) as xt_pool,
            tc.tile_pool(name="sb_big", bufs=1) as sb_big,
            tc.tile_pool(name="sb_small", bufs=2) as sb_small,
            tc.tile_pool(name="sb_scr", bufs=1) as sb_scr,
            tc.tile_pool(name="sb_rows", bufs=1) as sb_rows,
            tc.tile_pool(name="psA", bufs=5, space="PSUM") as psA,
            tc.tile_pool(name="ps_q", bufs=2, space="PSUM") as ps_q,
            tc.tile_pool(name="ps_b", bufs=1, space="PSUM") as ps_b,
        ):
            w = {}
            # critical-path loads first: adapter weight + bias + image-0 input
            w["w0t"] = wpool.tile([128, KT0, D], F32R, name="w0t_sb")
            for k in range(KT0):
                nc.sync.dma_start(out=w["w0t"][:, k, :],
                                  in_=dram["w0t_d"][k * 128:(k + 1) * 128, :])
            w["t0"] = wpool.tile([128, MT], F32, name="t0_sb")
            nc.gpsimd.dma_start(out=w["t0"], in_=dram["t0_d"].rearrange("(mt p) -> p mt", p=128))
            xt0 = xt_pool.tile([128, KT0, N], F32R, name="xt_sb")
            _load_xt(nc, dram, 0, xt0)

            # remaining weights load under image-0's adapter compute
            w["wrt"] = wpool.tile([128, MT, HID], F32R, name="wrt_sb")
            nc.sync.dma_start(out=w["wrt"], in_=dram["wrt_d"].rearrange("(kt p) m -> p kt m", p=128))
            w["wlt"] = wpool.tile([128, 9, HID], F32R, name="wlt_sb")
            nc.sync.dma_start(out=w["wlt"], in_=dram["wlt_d"].rearrange("t p m -> p t m"))
            w["wgt"] = wpool.tile([128, 9, HID], F32R, name="wgt_sb")
            nc.sync.dma_start(out=w["wgt"], in_=dram["wgt_d"].rearrange("t p m -> p t m"))
            w["wpt"] = wpool.tile([128, 2, D], F32R, name="wpt_sb")
            nc.sync.dma_start(out=w["wpt"], in_=dram["wpt_d"].rearrange("(kt p) m -> p kt m", p=128))
            w["wft"] = wpool.tile([128, MT, D], F32R, name="wft_sb")
            nc.sync.dma_start(out=w["wft"], in_=dram["wft_d"].rearrange("(kt p) m -> p kt m", p=128))
            w["pt"] = wpool.tile([128, MT, CK], F32R, name="pt_sb")
            nc.sync.dma_start(out=w["pt"], in_=dram["pt_d"].rearrange("(kt p) m -> p kt m", p=128))
            for nm, srcn, width in (("tp", "tp_d", MT), ("bf", "bf_d", MT),
                                    ("tr", "tr_d", 1), ("tl", "tl_d", 1), ("tg", "tg_d", 1)):
                w[nm] = wpool.tile([128, width], F32, name=f"{nm}_sb")
                nc.gpsimd.dma_start(out=w[nm], in_=dram[srcn].rearrange("(mt p) -> p mt", p=128))
            w["ones_r"] = wpool.tile([128, 2], F32R, name="ones_r")
            nc.gpsimd.dma_start(out=w["ones_r"], in_=dram["ones_d"].rearrange("(p two) -> p two", two=2))
            w["ones_row"] = wpool.tile([1, 128], F32R, name="ones_row")
            nc.gpsimd.dma_start(out=w["ones_row"], in_=dram["ones_d"][0:128].rearrange("(one p) -> one p", one=1))

            pools = (sb_big, sb_small, sb_scr, sb_rows, xt_pool, psA, ps_q, ps_b)
            _emit_image(nc, tc, pools, w, 0, dram, xt_sb=xt0)
            for b_idx in range(1, BPC):
                _emit_image(nc, tc, pools, w, b_idx, dram)

    nc.compile()
    return nc


_PROG = None


def _get_program():
    global _PROG
    if _PROG is None:
        _PROG = _build_program()
    return _PROG


def _fold(wc, g, b, m, v):
    s = g / np.sqrt(v + EPS)
    return wc * s[:, None, None, None], (b - m * s).astype(np.float32)


def kernel(patch_feats, protos, w0, g0, b0, m0, v0, wr, gr, br, mr, vr,
           wl, gl, bl, ml, vl, wg, gg, bg, mg, vg, wp, gp, bp, mp, vp,
           wf, bf, logit_scale):
    patch_feats = np.asarray(patch_feats, np.float32)
    f32 = lambda x: np.asarray(x, np.float32)

    w0f, t0 = _fold(f32(w0), f32(g0), f32(b0), f32(m0), f32(v0))
    wrf, tr = _fold(f32(wr), f32(gr), f32(br), f32(mr), f32(vr))
    wlf, tl = _fold(f32(wl), f32(gl), f32(bl), f32(ml), f32(vl))
    wgf, tg = _fold(f32(wg), f32(gg), f32(bg), f32(mg), f32(vg))
    wpf, tp = _fold(f32(wp), f32(gp), f32(bp), f32(mp), f32(vp))

    weights = {
        "w0t": np.ascontiguousarray(w0f[:, :, 0, 0].T),
        "wrt": np.ascontiguousarray(wrf[:, :, 0, 0].T),
        "wlt": np.ascontiguousarray(wlf.transpose(2, 3, 1, 0).reshape(9, HID, HID)),
        "wgt": np.ascontiguousarray(wgf.transpose(2, 3, 1, 0).reshape(9, HID, HID)),
        "wpt": np.ascontiguousarray(wpf[:, :, 0, 0].T),
        "wft": np.ascontiguousarray(f32(wf)[:, :, 0, 0].T),
        "pt": np.ascontiguousarray(f32(protos).reshape(CK, D).T),
        "t0": t0, "tr": tr, "tl": tl, "tg": tg, "tp": tp,
        "bf": f32(bf),
        "ones": np.ones(256, np.float32),
        "zeros": np.zeros(72, np.float32),
    }
    xt_all = np.ascontiguousarray(patch_feats.transpose(0, 2, 1))  # [B, 768, 1024]

    nc = _get_program()
    in_maps = []
    for c in range(NCORES):
        im = {"xt": xt_all[c * BPC:(c + 1) * BPC]}
        im.update(weights)
        in_maps.append(im)
    res = run_bass_kernel_spmd(nc, in_maps, list(range(NCORES)))

    araw = np.concatenate([res.results[c]["araw"] for c in range(NCORES)], axis=0)
    rnrow = np.concatenate([res.results[c]["rnrow"] for c in range(NCORES)], axis=0)
    x_cm = np.concatenate([res.results[c]["xout"] for c in range(NCORES)], axis=0)

    # araw: [B, 128, 8, C] with n = nt*128 + p -> [B, N, C]; apply rnorm scale
    a_nm = araw.transpose(0, 2, 1, 3).reshape(B, N, C)
    a_nm = a_nm * rnrow.reshape(B, N, 1)
    act_maps = np.ascontiguousarray(a_nm.transpose(0, 2, 1)).reshape(B, C, 32, 32).astype(np.float32)
    logits = (a_nm.max(axis=1) * float(np.asarray(logit_scale))).astype(np.float32)
    x = np.ascontiguousarray(x_cm.transpose(0, 2, 1)).astype(np.float32)
    return logits, act_maps, x


# revision 31
# speedup vs baseline: 1.2284x; 1.0307x over previous
"""TRN2 Bass kernel for nn_ClassNetPP (retrieval_knn).

Pipeline per image (channel-major activations [C, N] on-chip, N = H*W = 1024):
  adapter 1x1 conv + BN + ReLU -> ResidualContextBlock (1x1 reduce, 3x3 conv,
  3x3 dilated conv, 1x1 project + residual) -> final 1x1 conv + bias ->
  cosine sims vs 1280 prototypes -> max over K -> act maps + logits,
  plus L2-normalized embeddings.

Sharding: data-parallel over batch B=16 across 8 NeuronCores (2 images/core).
All matmuls run as fp32r (full PE rate at moving-dim >= 256, ~1e-4 rel err).
BN is folded into conv weights + per-channel bias on the host. The host also
pre-transposes inputs/weights so the device only does dense matmul-shaped work.
"""
import numpy as np

import concourse.bacc as bacc
import concourse.bass as bass
import concourse.mybir as mybir
import concourse.tile as tile
from concourse.bass_utils import run_bass_kernel_spmd
from concourse.masks import make_identity

F32 = mybir.dt.float32
F32R = mybir.dt.float32r

EPS = 1e-5
B, N, DIN, D, HID, C, K = 16, 1024, 768, 512, 128, 20, 64
CK = C * K  # 1280
NCORES = 8
BPC = B // NCORES  # images per core
KT0 = DIN // 128   # 6  k-tiles for adapter
MT = D // 128      # 4  m-tiles for D=512


def _r(ap):
    return ap.bitcast(F32R)


def _f(ap):
    return ap.bitcast(F32)


def _load_xt(nc, dram, b_idx, xt_sb):
    for ch in range(2):
        for k in range(KT0):
            nc.sync.dma_start(
                out=xt_sb[:, k, ch * 512:(ch + 1) * 512],
                in_=dram["xt"][b_idx, k * 128:(k + 1) * 128, ch * 512:(ch + 1) * 512],
            )


def _emit_image(nc, tc, pools, w, b_idx, dram, xt_sb=None):
    """Emit one image's pipeline. w: dict of SBUF weight APs; dram: dict of DRAM APs."""
    sb_big, sb_small, sb_scr, sb_rows, xt_pool, psA, ps_q = pools

    # ---- load transposed input [768, 1024] as 6 k-tiles (image 0's is preloaded)
    if xt_sb is None:
        xt_sb = xt_pool.tile([128, KT0, N], F32R, name="xt_sb")
        _load_xt(nc, dram, b_idx, xt_sb)

    # ---- L0: y0 = relu(w0t.T @ xt + t0)   [512, 1024]
    y0 = sb_big.tile([128, MT, N], F32R, name="y0")
    for ch in range(2):
        # k-outer over 4 live accumulators: the PE starts on k-tile 0 as soon
        # as its DMA lands instead of waiting for the whole input
        pss = [psA.tile([128, 512], F32, name="ps", tag="psA") for _ in range(MT)]
        for k in range(KT0):
            for mt in range(MT):
                nc.tensor.matmul(
                    pss[mt], w["w0t"][:, k, mt * 128:(mt + 1) * 128],
                    xt_sb[:, k, ch * 512:(ch + 1) * 512],
                    start=(k == 0), stop=(k == KT0 - 1), skip_group_check=True,
                )
        for mt in range(MT):
            nc.scalar.activation(
                out=y0[:, mt, ch * 512:(ch + 1) * 512], in_=pss[mt],
                func=mybir.ActivationFunctionType.Relu,
                bias=w["t0"][:, mt:mt + 1], scale=1.0,
            )

    # ---- Lr: o = relu(wrt.T @ y0 + tr) -> zero-padded [128, 36, 36]
    o_pad = sb_small.tile([128, 36, 36], F32R, name="o_pad")
    # zero the padding border via DMA from a zeros constant (DMA may write f32r)
    zb = dram["zeros_d"]
    nc.gpsimd.dma_start(out=o_pad[:, 0:2, :], in_=zb[0:72].partition_broadcast(128))
    nc.gpsimd.dma_start(out=o_pad[:, 34:36, :], in_=zb[0:72].partition_broadcast(128))
    nc.gpsimd.dma_start(out=o_pad[:, 2:34, 0:2], in_=zb[0:64].partition_broadcast(128))
    nc.gpsimd.dma_start(out=o_pad[:, 2:34, 34:36], in_=zb[0:64].partition_broadcast(128))
    for ch in range(2):
        ps = psA.tile([128, 512], F32, name="ps", tag="psA")
        for k in range(MT):
            nc.tensor.matmul(
                ps, w["wrt"][:, k, :], y0[:, k, ch * 512:(ch + 1) * 512],
                start=(k == 0), stop=(k == MT - 1),
            )
        # interior rows [2+16*ch : 2+16*(ch+1)], cols [2:34]
        nc.scalar.activation(
            out=o_pad[:, 2 + 16 * ch:2 + 16 * (ch + 1), 2:34], in_=ps,
            func=mybir.ActivationFunctionType.Relu,
            bias=w["tr"], scale=1.0,
        )

    # ---- 3x3 convs: l (dil=1, base offset 1) and g (dil=2, base offset 0)
    conv_outs = {}
    for cname, wname, bname, base, dil in (
        ("l_t", "wlt", "tl", 1, 1),
        ("g_t", "wgt", "tg", 0, 2),
    ):
        out_t = sb_small.tile([128, N], F32R, name=cname)
        for hc in range(2):  # 16 output rows per chunk
            ps = psA.tile([128, 512], F32, name="ps", tag="psA")
            for tap in range(9):
                dh, dw = tap // 3, tap % 3
                oh = base + dh * dil + hc * 16
                ow = base + dw * dil
                nc.tensor.matmul(
                    ps, w[wname][:, tap, :],
                    o_pad[:, oh:oh + 16, ow:ow + 32],
                    start=(tap == 0), stop=(tap == 8),
                )
            nc.scalar.activation(
                out=out_t[:, hc * 512:(hc + 1) * 512], in_=ps,
                func=mybir.ActivationFunctionType.Relu,
                bias=w[bname], scale=1.0,
            )
        conv_outs[cname] = out_t
    l_t, g_t = conv_outs["l_t"], conv_outs["g_t"]

    # ---- Lp + residual: x1 = relu(wpt.T @ [l;g] + tp + y0)
    x1 = sb_big.tile([128, MT, N], F32R, name="x1")
    for mt in range(MT):
        for ch in range(2):
            ps = psA.tile([128, 512], F32, name="ps", tag="psA")
            nc.tensor.matmul(ps, w["wpt"][:, 0, mt * 128:(mt + 1) * 128],
                             l_t[:, ch * 512:(ch + 1) * 512], start=True, stop=False)
            nc.tensor.matmul(ps, w["wpt"][:, 1, mt * 128:(mt + 1) * 128],
                             g_t[:, ch * 512:(ch + 1) * 512], start=False, stop=True)
            sl = x1[:, mt, ch * 512:(ch + 1) * 512]
            nc.vector.scalar_tensor_tensor(
                out=sl, in0=ps, scalar=w["tp"][:, mt:mt + 1],
                in1=_f(y0[:, mt, ch * 512:(ch + 1) * 512]),
                op0=mybir.AluOpType.add, op1=mybir.AluOpType.add,
            )
            nc.scalar.activation(out=sl, in_=_f(sl), func=mybir.ActivationFunctionType.Relu)

    # ---- Lf: xf = wft.T @ x1 + bf, with the ssq row-matmuls interleaved so
    # the PE never waits on the DVE square pass (pq accumulation groups span
    # the mt loop; other-bank matmuls interleave, which the HW allows).
    xf = sb_big.tile([128, MT, N], F32R, name="xf")
    sq = sb_scr.tile([128, MT, N], F32R, name="sq", tag="scr")
    pq0 = ps_q.tile([2, 512], F32, name="pq0", tag="pq")
    pq1 = ps_q.tile([2, 512], F32, name="pq1", tag="pq")
    pqs = (pq0, pq1)
    for mt in range(MT):
        for ch in range(2):
            ps = psA.tile([128, 512], F32, name="ps", tag="psA")
            for k in range(MT):
                nc.tensor.matmul(
                    ps, w["wft"][:, k, mt * 128:(mt + 1) * 128],
                    x1[:, k, ch * 512:(ch + 1) * 512],
                    start=(k == 0), stop=(k == MT - 1),
                )
            nc.scalar.activation(
                out=xf[:, mt, ch * 512:(ch + 1) * 512], in_=ps,
                func=mybir.ActivationFunctionType.Identity,
                bias=w["bf"][:, mt:mt + 1], scale=1.0,
            )
        nc.sync.dma_start(out=dram["xout"][b_idx, mt * 128:(mt + 1) * 128, :],
                          in_=_f(xf[:, mt, :]))
        nc.vector.tensor_mul(out=sq[:, mt, :], in0=_f(xf[:, mt, :]), in1=_f(xf[:, mt, :]))
        for ch in range(2):
            nc.tensor.matmul(
                pqs[ch], w["ones_r"], sq[:, mt, ch * 512:(ch + 1) * 512],
                start=(mt == 0), stop=(mt == MT - 1), skip_group_check=True,
            )

    # ---- rnorm_row = 1/max(sqrt(ssq), 1e-12) as [1, 1024] at partition 0
    srt = sb_rows.tile([1, N], F32, name="srt")
    for ch in range(2):
        nc.scalar.activation(out=srt[0:1, ch * 512:(ch + 1) * 512], in_=pqs[ch][0:1, :],
                             func=mybir.ActivationFunctionType.Sqrt)
    nc.vector.tensor_scalar_max(out=srt, in0=srt, scalar1=1e-12)
    nc.vector.reciprocal(out=srt, in_=srt)

    # ---- sims loop with interleaved broadcast / act-map transposes / x_norm
    # writeback, so every PE instruction's inputs are long since ready.
    a_nm = sb_small.tile([128, 8, C], F32, name="a_nm")
    chunks = [(0, 512, 8), (512, 512, 8), (1024, 256, 4)]  # (ck0, width, nclasses)

    def sims_nt(nt):
        c0 = 0
        for ck0, width, ncl in chunks:
            ps = psA.tile([128, 512], F32, name="ps", tag="psA")
            for k in range(MT):
                nc.tensor.matmul(
                    ps[:, :width], xf[:, k, nt * 128:(nt + 1) * 128],
                    w["pt"][:, k, ck0:ck0 + width],
                    start=(k == 0), stop=(k == MT - 1),
                )
            nc.vector.reduce_max(
                out=a_nm[:, nt, c0:c0 + ncl],
                in_=ps[:, :width].rearrange("p (c k) -> p c k", k=K),
                axis=mybir.AxisListType.X,
            )
            c0 += ncl

    def transpose_nt(nt):
        pst = psA.tile([128, 512], F32, name="ps", tag="psA")
        nc.tensor.transpose(pst[:C, :128], a_nm[:, nt, :], w["ident"])
        nc.scalar.copy(out=amap_sb[:, nt * 128:(nt + 1) * 128], in_=pst[:C, :128])

    for nt in range(8):
        sims_nt(nt)

    # raw act-map values (n-major) + rnorm row; the host applies the scale,
    # transposes to [C, H, W], and takes the max over n for logits
    nc.sync.dma_start(out=dram["araw"][b_idx], in_=a_nm)
    nc.sync.dma_start(out=dram["rnrow"][b_idx], in_=srt)


def _build_program():
    nc = bacc.Bacc("TRN2", target_bir_lowering=False, debug=False, num_devices=NCORES)

    dram = {
        "xt": nc.dram_tensor("xt", [BPC, DIN, N], F32R, kind="ExternalInput").ap(),
        "w0t_d": nc.dram_tensor("w0t", [DIN, D], F32R, kind="ExternalInput").ap(),
        "wrt_d": nc.dram_tensor("wrt", [D, HID], F32R, kind="ExternalInput").ap(),
        "wlt_d": nc.dram_tensor("wlt", [9, HID, HID], F32R, kind="ExternalInput").ap(),
        "wgt_d": nc.dram_tensor("wgt", [9, HID, HID], F32R, kind="ExternalInput").ap(),
        "wpt_d": nc.dram_tensor("wpt", [2 * HID, D], F32R, kind="ExternalInput").ap(),
        "wft_d": nc.dram_tensor("wft", [D, D], F32R, kind="ExternalInput").ap(),
        "pt_d": nc.dram_tensor("pt", [D, CK], F32R, kind="ExternalInput").ap(),
        "t0_d": nc.dram_tensor("t0", [D], F32, kind="ExternalInput").ap(),
        "tr_d": nc.dram_tensor("tr", [HID], F32, kind="ExternalInput").ap(),
        "tl_d": nc.dram_tensor("tl", [HID], F32, kind="ExternalInput").ap(),
        "tg_d": nc.dram_tensor("tg", [HID], F32, kind="ExternalInput").ap(),
        "tp_d": nc.dram_tensor("tp", [D], F32, kind="ExternalInput").ap(),
        "bf_d": nc.dram_tensor("bf", [D], F32, kind="ExternalInput").ap(),
        "ones_d": nc.dram_tensor("ones", [256], F32R, kind="ExternalInput").ap(),
        "zeros_d": nc.dram_tensor("zeros", [72], F32R, kind="ExternalInput").ap(),
        "xout": nc.dram_tensor("xout", [BPC, D, N], F32, kind="ExternalOutput").ap(),
        "araw": nc.dram_tensor("araw", [BPC, 128, 8, C], F32, kind="ExternalOutput").ap(),
        "rnrow": nc.dram_tensor("rnrow", [BPC, 1, N], F32, kind="ExternalOutput").ap(),
    }

    with tile.TileContext(nc) as tc:
        with (
            tc.tile_pool(name="wpool", bufs=1) as wpool,
            tc.tile_pool(name="xt_pool", bufs=1) as xt_pool,
            tc.tile_pool(name="sb_big", bufs=1) as sb_big,
            tc.tile_pool(name="sb_small", bufs=2) as sb_small,
            tc.tile_pool(name="sb_scr", bufs=1) as sb_scr,
            tc.tile_pool(name="sb_rows", bufs=1) as sb_rows,
            tc.tile_pool(name="psA", bufs=6, space="PSUM") as psA,
            tc.tile_pool(name="ps_q", bufs=2, space="PSUM") as ps_q,
        ):
            w = {}
            # critical-path loads first: adapter weight + bias + image-0 input
            w["w0t"] = wpool.tile([128, KT0, D], F32R, name="w0t_sb")
            for k in range(KT0):
                nc.sync.dma_start(out=w["w0t"][:, k, :],
                                  in_=dram["w0t_d"][k * 128:(k + 1) * 128, :])
            w["t0"] = wpool.tile([128, MT], F32, name="t0_sb")
            nc.gpsimd.dma_start(out=w["t0"], in_=dram["t0_d"].rearrange("(mt p) -> p mt", p=128))
            xt0 = xt_pool.tile([128, KT0, N], F32R, name="xt_sb")
            _load_xt(nc, dram, 0, xt0)

            # remaining weights load under image-0's adapter compute
            w["wrt"] = wpool.tile([128, MT, HID], F32R, name="wrt_sb")
            nc.sync.dma_start(out=w["wrt"], in_=dram["wrt_d"].rearrange("(kt p) m -> p kt m", p=128))
            w["wlt"] = wpool.tile([128, 9, HID], F32R, name="wlt_sb")
            nc.sync.dma_start(out=w["wlt"], in_=dram["wlt_d"].rearrange("t p m -> p t m"))
            w["wgt"] = wpool.tile([128, 9, HID], F32R, name="wgt_sb")
            nc.sync.dma_start(out=w["wgt"], in_=dram["wgt_d"].rearrange("t p m -> p t m"))
            w["wpt"] = wpool.tile([128, 2, D], F32R, name="wpt_sb")
            nc.sync.dma_start(out=w["wpt"], in_=dram["wpt_d"].rearrange("(kt p) m -> p kt m", p=128))
            w["wft"] = wpool.tile([128, MT, D], F32R, name="wft_sb")
            nc.sync.dma_start(out=w["wft"], in_=dram["wft_d"].rearrange("(kt p) m -> p kt m", p=128))
            w["pt"] = wpool.tile([128, MT, CK], F32R, name="pt_sb")
            nc.sync.dma_start(out=w["pt"], in_=dram["pt_d"].rearrange("(kt p) m -> p kt m", p=128))
            for nm, srcn, width in (("tp", "tp_d", MT), ("bf", "bf_d", MT),
                                    ("tr", "tr_d", 1), ("tl", "tl_d", 1), ("tg", "tg_d", 1)):
                w[nm] = wpool.tile([128, width], F32, name=f"{nm}_sb")
                nc.gpsimd.dma_start(out=w[nm], in_=dram[srcn].rearrange("(mt p) -> p mt", p=128))
            w["ones_r"] = wpool.tile([128, 2], F32R, name="ones_r")
            nc.gpsimd.dma_start(out=w["ones_r"], in_=dram["ones_d"].rearrange("(p two) -> p two", two=2))

            pools = (sb_big, sb_small, sb_scr, sb_rows, xt_pool, psA, ps_q)
            _emit_image(nc, tc, pools, w, 0, dram, xt_sb=xt0)
            for b_idx in range(1, BPC):
                _emit_image(nc, tc, pools, w, b_idx, dram)

    nc.compile()
    return nc


_PROG = None


def _get_program():
    global _PROG
    if _PROG is None:
        _PROG = _build_program()
    return _PROG


def _fold(wc, g, b, m, v):
    s = g / np.sqrt(v + EPS)
    return wc * s[:, None, None, None], (b - m * s).astype(np.float32)


def kernel(patch_feats, protos, w0, g0, b0, m0, v0, wr, gr, br, mr, vr,
           wl, gl, bl, ml, vl, wg, gg, bg, mg, vg, wp, gp, bp, mp, vp,
           wf, bf, logit_scale):
    patch_feats = np.asarray(patch_feats, np.float32)
    f32 = lambda x: np.asarray(x, np.float32)

    w0f, t0 = _fold(f32(w0), f32(g0), f32(b0), f32(m0), f32(v0))
    wrf, tr = _fold(f32(wr), f32(gr), f32(br), f32(mr), f32(vr))
    wlf, tl = _fold(f32(wl), f32(gl), f32(bl), f32(ml), f32(vl))
    wgf, tg = _fold(f32(wg), f32(gg), f32(bg), f32(mg), f32(vg))
    wpf, tp = _fold(f32(wp), f32(gp), f32(bp), f32(mp), f32(vp))

    weights = {
        "w0t": np.ascontiguousarray(w0f[:, :, 0, 0].T),
        "wrt": np.ascontiguousarray(wrf[:, :, 0, 0].T),
        "wlt": np.ascontiguousarray(wlf.transpose(2, 3, 1, 0).reshape(9, HID, HID)),
        "wgt": np.ascontiguousarray(wgf.transpose(2, 3, 1, 0).reshape(9, HID, HID)),
        "wpt": np.ascontiguousarray(wpf[:, :, 0, 0].T),
        "wft": np.ascontiguousarray(f32(wf)[:, :, 0, 0].T),
        "pt": np.ascontiguousarray(f32(protos).reshape(CK, D).T),
        "t0": t0, "tr": tr, "tl": tl, "tg": tg, "tp": tp,
        "bf": f32(bf),
        "ones": np.ones(256, np.float32),
        "zeros": np.zeros(72, np.float32),
    }
    xt_all = np.ascontiguousarray(patch_feats.transpose(0, 2, 1))  # [B, 768, 1024]

    nc = _get_program()
    in_maps = []
    for c in range(NCORES):
        im = {"xt": xt_all[c * BPC:(c + 1) * BPC]}
        im.update(weights)
        in_maps.append(im)
    res = run_bass_kernel_spmd(nc, in_maps, list(range(NCORES)))

    araw = np.concatenate([res.results[c]["araw"] for c in range(NCORES)], axis=0)
    rnrow = np.concatenate([res.results[c]["rnrow"] for c in range(NCORES)], axis=0)
    x_cm = np.concatenate([res.results[c]["xout"] for c in range(NCORES)], axis=0)

    # araw: [B, 128, 8, C] with n = nt*128 + p -> [B, N, C]; apply rnorm scale
    a_nm = araw.transpose(0, 2, 1, 3).reshape(B, N, C)
    a_nm = a_nm * rnrow.reshape(B, N, 1)
    act_maps = np.ascontiguousarray(a_nm.transpose(0, 2, 1)).reshape(B, C, 32, 32).astype(np.float32)
    logits = (a_nm.max(axis=1) * float(np.asarray(logit_scale))).astype(np.float32)
    x_cm = x_cm * rnrow.reshape(B, 1, N)
    x = np.ascontiguousarray(x_cm.transpose(0, 2, 1)).astype(np.float32)
    return logits, act_maps, x
